# revision 14
# baseline (speedup 1.0000x reference)
"""TRN2 Bass kernel for nn_MultiHeadAttention_78056735637728.

8-way data parallel over batch (B=8, one batch element per NeuronCore).
Host side: the padding mask m (shared across batch/heads/queries) is applied
EXACTLY by gathering only the unmasked kv positions — masked positions
contribute exp(logit - 1e9) == 0.0 in fp32 to every softmax, so dropping
them is bit-equivalent; k/v are gathered and zero-padded to a multiple of
128 and a per-position bias of -1e9 kills the pad rows inside the fused
exp on device.

Device kernel (per core), activations kept feature-major (transposed), all
matmul operands in float32r (TF32-class, 4x faster than fp32 on the PE):
  - Q.T/K.T projections with the weight blocks stationary; V projected into
    an interleaved [ones | V_h0 | V_h1 | ones] layout per kv tile.
  - logits.T (kv j on partitions, queries i free) via row-packed K=32
    matmuls: head pair (2hp, 2hp+1) at array rows 32*(h%4).
  - P = exp(logits/sqrt(32) + kvbias[j]) fused on ScalarE straight from
    PSUM (bottleneck engine: 1 elem/lane/cycle).
  - P@V and the softmax denominator fused per head into ONE 64-column
    stationary matmul: out rows [den_h0 | num_h0] (col group 0) and
    [num_h1 | den_h1] (col group 64) accumulated over kv tiles.
  - normalize on VectorE: gather dens -> reciprocal_approx_fast -> one
    tensor_mul over both heads' nums; every SBUF AP at base partition 0
    (HW silently drops partition-shifted DVE writes between SBUF APs).
  - output projection from four 64-row O.T tiles (K=64, one row group).
"""
import math
import sys
from contextlib import ExitStack

import numpy as np

for _p in ("/opt/trn_rl_repo", "/root/.axon_site/_ro/trn_rl_repo"):
    import os as _os
    if _os.path.isdir(_p) and _p not in sys.path:
        sys.path.insert(0, _p)
        break

import concourse.bass as bass  # noqa: E402
import concourse.tile as tile  # noqa: E402
from concourse import bacc, bass_utils, mybir  # noqa: E402
from concourse._compat import with_exitstack  # noqa: E402

F32 = mybir.dt.float32
F32R = mybir.dt.float32r
B = 8
S = 2048
D = 256
H = 8
DH = 32
PDIM = 128
N_CORES = 8

IN_NAMES = ["qt", "kt", "vt", "wq", "wk", "wv", "wo",
            "bq", "bk", "bo", "bvb", "kvb", "vne1"]


@with_exitstack
def _mha_kernel(ctx: ExitStack, tc: tile.TileContext, outs, ins, SKV, S=S):
    nc = tc.nc
    (i_qt, i_kt, i_vt, i_wq, i_wk, i_wv, i_wo,
     i_bq, i_bk, i_bo, i_bvb, i_kvb, i_vne1) = ins
    o_ot = outs[0]

    NJ = SKV // PDIM          # kv tiles of 128
    NQC = S // 512            # query chunks of 512
    assert S % 512 == 0 and SKV % PDIM == 0
    scale = 1.0 / math.sqrt(DH)

    def chunks(total, step=512):
        out = []
        c = 0
        while c < total:
            w = min(step, total - c)
            out.append((c, w))
            c += w
        return out

    consts = ctx.enter_context(tc.tile_pool(name="consts", bufs=1))

    # ---- load weights / biases / inputs (DRAM side already float32r) ----
    # critical-path loads on SP (HWDGE); bulk/non-critical on GPSIMD (SWDGE)
    def load2(ap_dram, cols, eng=None, step=512):
        eng = eng or nc.sync
        ts = []
        for b in range(2):
            t = consts.tile([PDIM, cols], F32R, name=f"{ap_dram.name}_sb{b}")
            for c0, w in chunks(cols, step):
                eng.dma_start(t[:, c0:c0 + w],
                              ap_dram[b * PDIM:(b + 1) * PDIM, c0:c0 + w])
            ts.append(t)
        return ts

    def load_bias(ap_dram, name, eng=None):
        t = consts.tile([PDIM, 2], F32, name=name)
        (eng or nc.sync).dma_start(
            t[:], ap_dram.rearrange("(b p) -> p b", p=PDIM))
        return t

    # SP queue: K-path first (bk, kvb, kt block 0), then the Q-path;
    # wk rides the GPSIMD queue in parallel
    wk_sb = load2(i_wk, D, eng=nc.gpsimd)
    bk_sb = load_bias(i_bk, "bk_sb")
    kvb_sb = consts.tile([PDIM, NJ], F32)
    nc.sync.dma_start(kvb_sb[:], i_kvb.rearrange("(j p) -> p j", p=PDIM))
    # tiny dummy activation so the exp table load happens at t~0, off the
    # critical path of the first real exp
    warm = consts.tile([PDIM, 1], F32, name="warm")
    nc.scalar.activation(warm[:], kvb_sb[:, 0:1],
                         mybir.ActivationFunctionType.Exp)

    acts = ctx.enter_context(tc.tile_pool(name="acts", bufs=1))
    QT = [acts.tile([PDIM, S], F32R, name=f"QT{b}") for b in range(2)]
    KT = [acts.tile([PDIM, SKV], F32R, name=f"KT{b}") for b in range(2)]
    # V in natural layout interleaved per head:
    # VNE[j] cols [h*64 : (h+1)*64] = [ones(32) | V_h(32)]
    VNE = [acts.tile([PDIM, 512], F32R, name=f"VNE{j}") for j in range(NJ)]
    # O.T as eight 32-row tiles so every normalize AP is at base partition 0
    OT = [acts.tile([32, S], F32R, name=f"OT{h}") for h in range(H)]
    wo8 = [acts.tile([32, D], F32R, name=f"wo8_{h}") for h in range(H)]

    # ---- projections (K fully; Q/V pipelined into the attention loop) ----
    proj_in = ctx.enter_context(tc.tile_pool(name="proj_in", bufs=1))

    def load2_split(ap_dram, cols, step=512):
        # block 0 chunks on SP/HWDGE, block 1 chunks on GPSIMD/SWDGE so the
        # two halves stream in parallel
        ts = []
        for b, eng in ((0, nc.sync), (1, nc.gpsimd)):
            t = proj_in.tile([PDIM, cols], F32R, name=f"{ap_dram.name}_sb{b}")
            for c0, w in chunks(cols, step):
                eng.dma_start(t[:, c0:c0 + w],
                              ap_dram[b * PDIM:(b + 1) * PDIM, c0:c0 + w])
            ts.append(t)
        return ts

    kt_sb = load2_split(i_kt, SKV)
    bvb_sb = consts.tile([PDIM, D], F32)
    nc.gpsimd.dma_start(bvb_sb[:], i_bvb[:])
    wv_sb = load2(i_wv, D, eng=nc.gpsimd)
    vt_sb = load2(i_vt, SKV, eng=nc.scalar)
    wq_sb = load2(i_wq, D)
    bq_sb = load_bias(i_bq, "bq_sb")
    qt_sb = load2_split(i_qt, S)
    for h in range(H):
        nc.gpsimd.dma_start(wo8[h][:], i_wo[h * DH:(h + 1) * DH, :])
    bo_sb = load_bias(i_bo, "bo_sb", eng=nc.gpsimd)

    def proj_qk(dst, w_sb, b_sb, x_sb, c0, w, pool):
        for ob in range(2):
            ps = pool.tile([PDIM, 512], F32, tag="ops")
            for ib in range(2):
                nc.tensor.matmul(
                    ps[:, 0:w],
                    lhsT=w_sb[ib][:, ob * PDIM:(ob + 1) * PDIM],
                    rhs=x_sb[ib][:, c0:c0 + w],
                    start=(ib == 0), stop=(ib == 1),
                )
            nc.vector.tensor_scalar_add(
                dst[ob][:, c0:c0 + w], ps[:, 0:w], b_sb[:, ob:ob + 1])

    def proj_v(j, pool):
        ps = pool.tile([PDIM, D], F32, tag="ops")
        for ib in range(2):
            nc.tensor.matmul(
                ps[:],
                lhsT=vt_sb[ib][:, j * PDIM:(j + 1) * PDIM],
                rhs=wv_sb[ib][:],
                start=(ib == 0), stop=(ib == 1),
            )
        # ones into the leading 32-col block per head, V+bias after
        v3 = VNE[j][:].rearrange("p (h c) -> p h c", c=2 * DH)
        nc.sync.dma_start(
            v3[:, :, 0:DH],
            i_vne1.rearrange("p (h c) -> p h c", c=DH))
        nc.vector.tensor_add(
            v3[:, :, DH:2 * DH],
            ps[:].rearrange("p (h c) -> p h c", c=DH),
            bvb_sb[:].rearrange("p (h c) -> p h c", c=DH))

    with tc.tile_pool(name="proj_ps", bufs=2, space="PSUM") as proj_ps:
        for c0, w in chunks(SKV):
            proj_qk(KT, wk_sb, bk_sb, kt_sb, c0, w, proj_ps)
        proj_qk(QT, wq_sb, bq_sb, qt_sb, 0, 512, proj_ps)

    # ---- attention + output projection (per query chunk) ----
    with tc.tile_pool(name="lps", bufs=2, space="PSUM") as lps_pool, \
         tc.tile_pool(name="pvps", bufs=1, space="PSUM") as pv_pool, \
         tc.tile_pool(name="ops", bufs=2, space="PSUM") as out_ps, \
         tc.tile_pool(name="psb", bufs=3) as p_pool, \
         tc.tile_pool(name="osb", bufs=2) as out_sb, \
         tc.tile_pool(name="norm", bufs=2) as norm_pool:
        def outproj(c):
            i0c = c * 512
            for ob in range(2):
                ps = out_ps.tile([PDIM, 512], F32, tag="ops")
                for h in range(H):
                    nc.tensor.matmul(
                        ps[:],
                        lhsT=wo8[h][:, ob * PDIM:(ob + 1) * PDIM],
                        rhs=OT[h][:, i0c:i0c + 512],
                        start=(h == 0), stop=(h == H - 1),
                    )
                ft = out_sb.tile([PDIM, 512], F32, tag="ft")
                nc.vector.tensor_scalar_add(ft[:], ps[:], bo_sb[:, ob:ob + 1])
                nc.gpsimd.dma_start(
                    o_ot[ob * PDIM:(ob + 1) * PDIM, i0c:i0c + 512],
                    ft[:])

        def emit_pv(pva, pvb, h0, h1, j, pt):
            # fused PV + denominator per head, each into its own 1-bank
            # PSUM tile: rows [den_h | num_h] at base 0
            for pvt, h in ((pva, h0), (pvb, h1)):
                nc.tensor.matmul(
                    pvt[:],
                    lhsT=VNE[j][:, h * 2 * DH:(h + 1) * 2 * DH],
                    rhs=pt[:, (h - h0) * 512:(h - h0 + 1) * 512],
                    start=(j == 0), stop=(j == NJ - 1),
                    tile_position=(0, 0),
                    skip_group_check=True,
                )

        def emit_norm(pva, pvb, h0, h1, i0):
            # normalize: pv rows = [den_h | num_h], all SBUF APs at base 0
            for pvt, h in ((pva, h0), (pvb, h1)):
                rec = norm_pool.tile([32, 512], F32, tag="rec")
                nc.vector.reciprocal_approx_fast(rec[:], pvt[0:32, :])
                nc.vector.tensor_mul(
                    OT[h][:, i0:i0 + 512], pvt[32:64, :], rec[:])

        pending = None  # previous group's last PV + normalize, emitted after
        # the next group's first logits+exp so the PE stream keeps ScalarE fed
        for ic in range(NQC):
            i0 = ic * 512
            for hp in range(4):
                h0, h1 = 2 * hp, 2 * hp + 1
                pva = pv_pool.tile([64, 512], F32, tag="pva")
                pvb = pv_pool.tile([64, 512], F32, tag="pvb")
                for j in range(NJ):
                    if ic == 0 and hp == 0:
                        proj_v(j, out_ps)
                    # spread next-chunk Q projection / previous-chunk output
                    # projection into the PE slack of later head groups
                    if j == 0 and hp == 1 and ic + 1 < NQC:
                        proj_qk(QT, wq_sb, bq_sb, qt_sb,
                                (ic + 1) * 512, 512, out_ps)
                    if j == 0 and hp == 2 and ic > 0:
                        outproj(ic - 1)
                    lt = lps_pool.tile([PDIM, 1024], F32, tag="lt")
                    for hh, h in enumerate((h0, h1)):
                        t = h // 4
                        bp = 32 * (h % 4)
                        nc.tensor.matmul(
                            lt[:, hh * 512:(hh + 1) * 512],
                            lhsT=KT[t][bp:bp + 32, j * PDIM:(j + 1) * PDIM],
                            rhs=QT[t][bp:bp + 32, i0:i0 + 512],
                            start=True, stop=True,
                            tile_position=(bp, 0),
                        )
                    pt = p_pool.tile([PDIM, 1024], F32R, tag="pt")
                    nc.scalar.activation(
                        pt[:], lt[:], mybir.ActivationFunctionType.Exp,
                        bias=kvb_sb[:, j:j + 1], scale=scale)
                    if pending is not None:
                        pending()
                        pending = None
                    if j < NJ - 1:
                        emit_pv(pva, pvb, h0, h1, j, pt)
                    else:
                        def pending(pva=pva, pvb=pvb, h0=h0, h1=h1, j=j,
                                    pt=pt, i0=i0):
                            emit_pv(pva, pvb, h0, h1, j, pt)
                            emit_norm(pva, pvb, h0, h1, i0)

        pending()
        outproj(NQC - 1)


_PROGRAM_CACHE = {}

# DRAM dtypes: matmul operands land as float32r (same 4-byte payload; the
# PE rounds internally), everything else float32
_F32R_INPUTS = {"qt", "kt", "vt", "wq", "wk", "wv", "wo", "vne1"}


def _make_program(SKV, S=S):
    nc = bacc.Bacc("TRN2", target_bir_lowering=False, debug=False,
                   enable_asserts=False, num_devices=1)
    shapes = dict(qt=(D, S), kt=(D, SKV), vt=(D, SKV), wq=(D, D), wk=(D, D),
                  wv=(D, D), wo=(D, D), bq=(D,), bk=(D,), bo=(D,),
                  bvb=(PDIM, D), kvb=(SKV,), vne1=(PDIM, D))
    in_aps = [nc.dram_tensor(k, shapes[k],
                             F32R if k in _F32R_INPUTS else F32,
                             kind="ExternalInput").ap()
              for k in IN_NAMES]
    out_ap = nc.dram_tensor("ot", (D, S), F32, kind="ExternalOutput").ap()
    with tile.TileContext(nc) as tc:
        _mha_kernel(tc, [out_ap], in_aps, SKV=SKV, S=S)
    nc.compile()
    return nc


def _get_program(SKV):
    if SKV not in _PROGRAM_CACHE:
        _PROGRAM_CACHE[SKV] = _make_program(SKV)
    return _PROGRAM_CACHE[SKV]


def _prepare_in_maps(q, k, v, m, wq, bq, wk, bk, wv, bv, wo, bo):
    mask = np.asarray(m, np.float32).reshape(-1)
    keep = np.flatnonzero(mask == 0.0)
    skv = len(keep)
    assert skv > 0, "all kv positions masked"
    SKV = max(PDIM, ((skv + PDIM - 1) // PDIM) * PDIM)

    kvb = np.zeros(SKV, np.float32)
    kvb[skv:] = -1e9
    bvb = np.ascontiguousarray(np.tile(np.asarray(bv, np.float32)[None, :],
                                       (PDIM, 1)))
    common = dict(
        wq=np.ascontiguousarray(wq, np.float32),
        wk=np.ascontiguousarray(wk, np.float32),
        wv=np.ascontiguousarray(wv, np.float32),
        wo=np.ascontiguousarray(wo, np.float32),
        bq=np.ascontiguousarray(bq, np.float32),
        bk=np.ascontiguousarray(bk, np.float32),
        bo=np.ascontiguousarray(bo, np.float32),
        bvb=bvb, kvb=kvb,
        vne1=np.ones((PDIM, D), np.float32),
    )
    in_maps = []
    for b in range(B):
        kg = np.zeros((D, SKV), np.float32)
        vg = np.zeros((D, SKV), np.float32)
        kg[:, :skv] = np.asarray(k[b], np.float32).T[:, keep]
        vg[:, :skv] = np.asarray(v[b], np.float32).T[:, keep]
        in_maps.append(dict(
            qt=np.ascontiguousarray(np.asarray(q[b], np.float32).T),
            kt=kg, vt=vg, **common))
    return in_maps, SKV


def _run(q, k, v, m, wq, bq, wk, bk, wv, bv, wo, bo, trace=False):
    in_maps, SKV = _prepare_in_maps(q, k, v, m, wq, bq, wk, bk, wv, bv, wo, bo)
    nc = _get_program(SKV)
    last_err = None
    for attempt in range(3):
        try:
            res = bass_utils.run_bass_kernel_spmd(
                nc, in_maps, core_ids=list(range(N_CORES)), trace=trace)
            break
        except Exception as e:  # transient device-unrecoverable states heal
            last_err = e        # on the next NEFF load; retry
    else:
        raise last_err
    out = np.stack([res.results[b]["ot"].T for b in range(B)], axis=0)
    return np.ascontiguousarray(out, np.float32), res


def kernel(q, k, v, m, wq, bq, wk, bk, wv, bv, wo, bo):
    out, _ = _run(q, k, v, m, wq, bq, wk, bk, wv, bv, wo, bo, trace=False)
    return out
